# revision 17
# baseline (speedup 1.0000x reference)
"""MultiHeadAttention Trainium2 kernel, tensor-parallel over heads on 8 NeuronCores.

Contract: kernel(**inputs) takes FULL inputs (x, Wq, Wk, Wv, Wo, bo) and
returns the FULL outputs (y, att) matching reference.py.

Sharding: 16 heads / 8 cores = 2 heads per core. Wq/Wk/Wv are sliced
column-wise by head (rows of the torch-Linear weight), Wo row-wise.
Per-core partial y outputs are summed on the host; att head-shards are
concatenated on the host.

Per-core dataflow (all matmuls in float32r: fp32 storage, ~1e-4 rel err,
full PE speed):
  stage 1: qT/kT/vT [128ch, 4096tok] = W_cT.T @ xT   (x.T uploaded by host)
  stage 2: vT -> v_aug [tok, (v_h0|1|v_h1|1)] via PE transposes
  pass A (per head, [k,q] layout): weiT = kT.T@qT -> expT (ACT) ->
          yT[65,q] += v_aug.T @ expT  (row 64 = softmax denominator S)
  pass B ([q,k] layout): wei = qT.T@kT -> att = exp(wei - lnS) via ACT
          per-partition bias -> DMA straight to DRAM (normalized).
  stage C: ycT *= 1/S (K=2 mask-broadcast matmul + DVE mul),
          y_partial = ycT.T @ WoT -> DRAM.
"""

import contextlib

import numpy as np

import concourse.bacc as bacc
import concourse.tile as tile
from concourse import mybir
from concourse.bass import ts
from concourse.bass_utils import run_bass_kernel_spmd
from concourse.masks import make_identity

F32 = mybir.dt.float32
F32R = mybir.dt.float32r
Exp = mybir.ActivationFunctionType.Exp
Ln = mybir.ActivationFunctionType.Ln

B, T, D, H = 2, 2048, 1024, 16
HS = D // H            # 64  head size
NCORES = 8
HPC = H // NCORES      # 2   heads per core
CH = HPC * HS          # 128 channels per core
BT = B * T             # 4096 tokens

LAST_EXEC_NS = None
LAST_RESULT = None
_NC_CACHE = None


def _build():
    nc = bacc.Bacc(None)

    xT = nc.dram_tensor("xT", [D, BT], F32R, kind="ExternalInput")
    wqT = nc.dram_tensor("wqT", [D, CH], F32R, kind="ExternalInput")
    wkT = nc.dram_tensor("wkT", [D, CH], F32R, kind="ExternalInput")
    wvT = nc.dram_tensor("wvT", [D, CH], F32R, kind="ExternalInput")
    woT = nc.dram_tensor("woT", [CH, D], F32R, kind="ExternalInput")
    hmask = nc.dram_tensor("hmask", [HPC, 128], F32R, kind="ExternalInput")
    att_out = nc.dram_tensor("att", [B, HPC, T, T], F32, kind="ExternalOutput")
    y_out = nc.dram_tensor("y", [BT, D], F32, kind="ExternalOutput")

    NKT = T // 128         # 16 k tiles per batch
    NQT = T // 128         # 16 q tiles per batch
    NBKT = BT // 128       # 32 k tiles total

    with tile.TileContext(nc) as tc, contextlib.ExitStack() as ctx:
        pp = ctx.enter_context(tc.tile_pool(name="persist", bufs=1))
        xp = ctx.enter_context(tc.tile_pool(name="xp", bufs=2))
        sbv = ctx.enter_context(tc.tile_pool(name="sbv", bufs=2))
        sbe = ctx.enter_context(tc.tile_pool(name="sbe", bufs=3))
        sba = ctx.enter_context(tc.tile_pool(name="sba", bufs=3))
        sbg = ctx.enter_context(tc.tile_pool(name="sbg", bufs=2))
        sbr = ctx.enter_context(tc.tile_pool(name="sbr", bufs=1))
        sby = ctx.enter_context(tc.tile_pool(name="sby", bufs=2))
        psw = ctx.enter_context(tc.tile_pool(name="psw", bufs=3, space="PSUM"))
        psy = ctx.enter_context(tc.tile_pool(name="psy", bufs=1, space="PSUM"))

        qT = pp.tile([CH, BT], F32R, tag="qT")
        kT = pp.tile([CH, BT], F32R, tag="kT")
        wq_sb = pp.tile([128, 8, 128], F32R, tag="wq")
        wk_sb = pp.tile([128, 8, 128], F32R, tag="wk")
        wv_sb = pp.tile([128, 8, 128], F32R, tag="wv")
        wo_sb = pp.tile([CH, D], F32R, tag="wo")
        ident = pp.tile([128, 128], F32, tag="ident")
        hmask_sb = pp.tile([HPC, 128], F32R, tag="hmask")
        v_aug = pp.tile([128, NBKT, 2 * (HS + 1)], F32R, tag="v_aug")
        ones_sb = pp.tile([128, NBKT], F32, tag="ones")
        negLnS = pp.tile([128, NQT, B, HPC], F32, tag="negLnS")
        ycT0 = sbr.tile([CH, T], F32R, tag="ycT0", name="ycT0")
        ycT1 = sbr.tile([CH, T], F32R, tag="ycT1", name="ycT1")
        S_b = sbr.tile([HPC, T], F32, tag="Sb", name="S_b")
        recipSr_b = sbr.tile([HPC, T], F32R, tag="recipSb", name="recipSr_b")
        rawST = sbr.tile([128, NQT, HPC], F32, tag="rawST", name="rawST")
        ycT_b = {0: ycT0, 1: ycT1}

        make_identity(nc, ident[:])
        nc.sync.dma_start(out=hmask_sb, in_=hmask[:, :])
        nc.sync.dma_start(out=wo_sb, in_=woT[:, :])
        for w_sb, w_dr in ((wq_sb, wqT), (wk_sb, wkT), (wv_sb, wvT)):
            nc.sync.dma_start(
                out=w_sb[:],
                in_=w_dr[:, :].rearrange("(dt p) c -> p dt c", p=128),
            )
        nc.vector.memset(ones_sb[:], 1.0)
        nc.vector.tensor_copy(v_aug[:, :, HS], ones_sb[:])
        nc.vector.tensor_copy(v_aug[:, :, 2 * HS + 1], ones_sb[:])

        # x DMAs all emitted upfront; xp bufs=2 paces them against consumption.
        x_tiles = []
        for e in range(8):
            x_sb = xp.tile([128, 8, 512], F32R, tag="xq", name=f"x{e}")
            for dt in range(8):
                nc.sync.dma_start(
                    out=x_sb[:, dt, :],
                    in_=xT[dt * 128:(dt + 1) * 128, e * 512:(e + 1) * 512],
                )
            x_tiles.append(x_sb)

        vT_es = {}

        _accflip = [0]

        def proj_group(e, w_sb, dest, off):
            def f():
                _accflip[0] ^= 1
                acc = psy.tile([128, 512], F32, tag=f"yT{_accflip[0]}", name="acc")
                for dt in range(8):
                    nc.tensor.matmul(
                        acc[:],
                        w_sb[:, dt, :],
                        x_tiles[e][:, dt, :],
                        start=(dt == 0),
                        stop=(dt == 7),
                    )
                nc.vector.tensor_copy(dest[:, off:off + 512], acc[:])
            return f

        def trans_group(e):
            def f():
                vT_e = vT_es[e]
                for j in range(4):
                    bkt = e * 4 + j
                    _accflip[0] ^= 1
                    tp = psy.tile([128, 512], F32, tag=f"yT{_accflip[0]}", name="tp")
                    nc.tensor.transpose(
                        tp[:, 0:128], vT_e[:, ts(j, 128)].bitcast(F32), ident[:]
                    )
                    nc.vector.tensor_copy(v_aug[:, bkt, 0:HS], tp[:, 0:HS])
                    nc.vector.tensor_copy(
                        v_aug[:, bkt, HS + 1:2 * HS + 1], tp[:, HS:2 * HS]
                    )
            return f

        def eighth_fillers(e):
            vT_es[e] = sbv.tile([CH, 512], F32R, tag="vTe", name=f"vTe{e}")
            return [
                proj_group(e, wq_sb, qT, e * 512),
                proj_group(e, wk_sb, kT, e * 512),
                proj_group(e, wv_sb, vT_es[e], 0),
                trans_group(e),
            ]

        def emit_pass_a(b):
            """Heads packed in PE row groups: unnormalized y + S (+S transposes)."""
            ycT = ycT_b[b]
            pending_strans = []

            def strans(qts):
                def f():
                    for qt in qts:
                        _accflip[0] ^= 1
                        stp = psy.tile([128, 512], F32, tag=f"yT{_accflip[0]}", name="stp")
                        nc.tensor.transpose(
                            stp[:, 0:HPC],
                            S_b[0:HPC, qt * 128:(qt + 1) * 128],
                            ident[0:HPC, 0:HPC],
                        )
                        nc.vector.tensor_copy(rawST[:, qt, :], stp[:, 0:HPC])
                return f

            for qc in range(4):  # 512-wide q chunks
                q0 = b * T + qc * 512
                yT0 = psy.tile([HS + 1, 512], F32, tag="yT0", name="yT0")
                yT1 = psy.tile([HS + 1, 512], F32, tag="yT1", name="yT1")
                for kt in range(NKT):
                    k0 = b * T + kt * 128
                    wei = psw.tile([128, 1024], F32, tag="w")
                    nc.tensor.matmul(
                        wei[:, 0:512],
                        kT[0:HS, k0:k0 + 128],
                        qT[0:HS, q0:q0 + 512],
                        start=True, stop=True,
                    )
                    nc.tensor.matmul(
                        wei[:, 512:1024],
                        kT[HS:2 * HS, k0:k0 + 128],
                        qT[HS:2 * HS, q0:q0 + 512],
                        start=True, stop=True,
                    )
                    expT = sbe.tile([128, 1024], F32R, tag="expT")
                    nc.scalar.activation(expT[:], wei[:], Exp)
                    nc.tensor.matmul(
                        yT0[:],
                        v_aug[:, b * NKT + kt, 0:HS + 1],
                        expT[:, 0:512],
                        start=(kt == 0), stop=(kt == NKT - 1),
                    )
                    nc.tensor.matmul(
                        yT1[:],
                        v_aug[:, b * NKT + kt, HS + 1:2 * (HS + 1)],
                        expT[:, 512:1024],
                        start=(kt == 0), stop=(kt == NKT - 1),
                    )
                for h, yT_ps in ((0, yT0), (1, yT1)):
                    stage65 = sbg.tile([HS + 1, 512], F32, tag="st65")
                    nc.vector.tensor_copy(stage65[:], yT_ps[:])
                    if h == 0:
                        nc.vector.tensor_copy(
                            ycT[0:HS, qc * 512:(qc + 1) * 512], yT_ps[0:HS, :]
                        )
                    else:
                        nc.gpsimd.dma_start(
                            out=ycT[HS:2 * HS, qc * 512:(qc + 1) * 512],
                            in_=stage65[0:HS, :],
                        )
                    nc.gpsimd.dma_start(
                        out=S_b[h:h + 1, qc * 512:(qc + 1) * 512],
                        in_=stage65[HS:HS + 1, :],
                    )
            strans(range(0, 16))()

        def emit_sprep_c(b):
            """-ln(S) bias prep + 1/S scaling of ycT (recipS = exp(-ln S) on ACT)."""
            ycT = ycT_b[b]
            nc.scalar.activation(negLnS[:, :, b, :], rawST[:], Ln)
            lnS_b = sba.tile([HPC, T], F32, tag="att", name="lnS_b")
            nc.scalar.activation(lnS_b[:], S_b[:], Ln)
            nc.vector.tensor_scalar_mul(
                negLnS[:, :, b, :], negLnS[:, :, b, :], -1.0
            )
            nc.scalar.activation(recipSr_b[:], lnS_b[:], Exp, scale=-1.0)
            for c2 in range(2):  # 1024-token chunks within b
                bc = psw.tile([128, 1024], F32, tag="w")
                for n in range(2):
                    nc.tensor.matmul(
                        bc[:, ts(n, 512)],
                        hmask_sb[:],
                        recipSr_b[0:HPC, c2 * 1024 + n * 512:
                                  c2 * 1024 + (n + 1) * 512],
                        start=True, stop=True,
                    )
                nc.vector.tensor_mul(
                    ycT[:, ts(c2, 1024)], ycT[:, ts(c2, 1024)], bc[:]
                )
            for tt in range(16):  # out-projection token tiles of this b
                tok = b * T + tt * 128
                op = psw.tile([128, 1024], F32, tag="w", name="op")
                for n in range(2):
                    nc.tensor.matmul(
                        op[:, ts(n, 512)],
                        ycT[:, tt * 128:(tt + 1) * 128],
                        wo_sb[:, ts(n, 512)],
                        start=True, stop=True,
                    )
                y_sb = sby.tile([128, D], F32, tag="y")
                nc.vector.tensor_copy(y_sb[:], op[:])
                nc.sync.dma_start(out=y_out[tok:tok + 128, :], in_=y_sb[:])

        def emit_pass_b(b, fillers):
            """Normalized att via exp(wei - lnS); out-projection interleaved."""
            ycT = ycT_b[b]
            for qt in range(NQT):
                qs = b * T + qt * 128
                att0 = sba.tile([128, T], F32, tag="att")
                att1 = sba.tile([128, T], F32, tag="att")
                for kc in range(2):  # 1024-wide k chunks
                    wei0 = psw.tile([128, 1024], F32, tag="w")
                    wei1 = psw.tile([128, 1024], F32, tag="w")
                    for n in range(2):
                        ks = b * T + kc * 1024 + n * 512
                        nc.tensor.matmul(
                            wei0[:, ts(n, 512)],
                            qT[0:HS, qs:qs + 128],
                            kT[0:HS, ks:ks + 512],
                            start=True, stop=True,
                        )
                        nc.tensor.matmul(
                            wei1[:, ts(n, 512)],
                            qT[HS:2 * HS, qs:qs + 128],
                            kT[HS:2 * HS, ks:ks + 512],
                            start=True, stop=True,
                        )
                    nc.scalar.activation(
                        att0[:, ts(kc, 1024)], wei0[:], Exp,
                        bias=negLnS[:, qt, b, 0:1],
                    )
                    nc.scalar.activation(
                        att1[:, ts(kc, 1024)], wei1[:], Exp,
                        bias=negLnS[:, qt, b, 1:2],
                    )
                    if kc == 0 and fillers:
                        fillers.pop(0)()
                nc.sync.dma_start(out=att_out[b, 0, ts(qt, 128), :], in_=att0[:])
                nc.sync.dma_start(out=att_out[b, 1, ts(qt, 128), :], in_=att1[:])

        # ---- emission ----
        for e in range(8):
            for f in eighth_fillers(e):
                f()
        emit_pass_a(0)
        emit_sprep_c(0)
        emit_pass_b(0, [])
        emit_pass_a(1)
        emit_sprep_c(1)
        emit_pass_b(1, [])

    nc.finalize()
    return nc


def kernel(x, Wq, Wk, Wv, Wo, bo, _trace=False, _tmpdir=None):
    global LAST_EXEC_NS, LAST_RESULT, _NC_CACHE
    x = np.asarray(x, dtype=np.float32)
    Wq = np.asarray(Wq, dtype=np.float32)
    Wk = np.asarray(Wk, dtype=np.float32)
    Wv = np.asarray(Wv, dtype=np.float32)
    Wo = np.asarray(Wo, dtype=np.float32)
    bo = np.asarray(bo, dtype=np.float32)

    scale = 1.0 / np.sqrt(np.float32(HS))
    xT_host = np.ascontiguousarray(x.reshape(BT, D).T)
    hm = np.zeros((HPC, 128), np.float32)
    for h in range(HPC):
        hm[h, h * HS:(h + 1) * HS] = 1.0

    in_maps = []
    for c in range(NCORES):
        cs = slice(c * CH, (c + 1) * CH)
        in_maps.append({
            "xT": xT_host,
            "wqT": np.ascontiguousarray(Wq[cs, :].T) * scale,
            "wkT": np.ascontiguousarray(Wk[cs, :].T),
            "wvT": np.ascontiguousarray(Wv[cs, :].T),
            "woT": np.ascontiguousarray(Wo[:, cs].T),
            "hmask": hm,
        })

    if _NC_CACHE is None:
        _NC_CACHE = _build()
    nc = _NC_CACHE

    res = run_bass_kernel_spmd(
        nc, in_maps, core_ids=list(range(NCORES)), trace=_trace, tmpdir=_tmpdir
    )
    LAST_EXEC_NS = res.exec_time_ns
    LAST_RESULT = res

    att = np.empty((B, H, T, T), dtype=np.float32)
    y = np.zeros((BT, D), dtype=np.float64)
    for c in range(NCORES):
        att[:, c * HPC:(c + 1) * HPC] = res.results[c]["att"]
        y += res.results[c]["y"]
    y = (y + bo).astype(np.float32).reshape(B, T, D)
    return y, att


# revision 18
# speedup vs baseline: 1.0947x; 1.0947x over previous
"""MultiHeadAttention Trainium2 kernel, tensor-parallel over heads on 8 NeuronCores.

Contract: kernel(**inputs) takes FULL inputs (x, Wq, Wk, Wv, Wo, bo) and
returns the FULL outputs (y, att) matching reference.py.

Sharding: 16 heads / 8 cores = 2 heads per core. Wq/Wk/Wv are sliced
column-wise by head (rows of the torch-Linear weight), Wo row-wise.
Per-core partial y outputs are summed on the host; att head-shards are
concatenated on the host.

Per-core dataflow (all matmuls in float32r: fp32 storage, ~1e-4 rel err,
full PE speed):
  stage 1: qT/kT/vT [128ch, 4096tok] = W_cT.T @ xT   (x.T uploaded by host)
  stage 2: vT -> v_aug [tok, (v_h0|1|v_h1|1)] via PE transposes
  pass A (per head, [k,q] layout): weiT = kT.T@qT -> expT (ACT) ->
          yT[65,q] += v_aug.T @ expT  (row 64 = softmax denominator S)
  pass B ([q,k] layout): wei = qT.T@kT -> att = exp(wei - lnS) via ACT
          per-partition bias -> DMA straight to DRAM (normalized).
  stage C: ycT *= 1/S (K=2 mask-broadcast matmul + DVE mul),
          y_partial = ycT.T @ WoT -> DRAM.
"""

import contextlib

import numpy as np

import concourse.bacc as bacc
import concourse.tile as tile
from concourse import mybir
from concourse.bass import ts
from concourse.bass_utils import run_bass_kernel_spmd
from concourse.masks import make_identity

F32 = mybir.dt.float32
F32R = mybir.dt.float32r
Exp = mybir.ActivationFunctionType.Exp
Ln = mybir.ActivationFunctionType.Ln

B, T, D, H = 2, 2048, 1024, 16
HS = D // H            # 64  head size
NCORES = 8
HPC = H // NCORES      # 2   heads per core
CH = HPC * HS          # 128 channels per core
BT = B * T             # 4096 tokens

LAST_EXEC_NS = None
LAST_RESULT = None
_NC_CACHE = None


def _build():
    nc = bacc.Bacc(None)

    xT = nc.dram_tensor("xT", [D, BT], F32R, kind="ExternalInput")
    wqT = nc.dram_tensor("wqT", [D, CH], F32R, kind="ExternalInput")
    wkT = nc.dram_tensor("wkT", [D, CH], F32R, kind="ExternalInput")
    wvT = nc.dram_tensor("wvT", [D, CH], F32R, kind="ExternalInput")
    woT = nc.dram_tensor("woT", [CH, D], F32R, kind="ExternalInput")
    hmask = nc.dram_tensor("hmask", [HPC, 128], F32R, kind="ExternalInput")
    att_out = nc.dram_tensor("att", [B, HPC, T, T], F32, kind="ExternalOutput")
    y_out = nc.dram_tensor("y", [BT, D], F32, kind="ExternalOutput")

    NKT = T // 128         # 16 k tiles per batch
    NQT = T // 128         # 16 q tiles per batch
    NBKT = BT // 128       # 32 k tiles total

    with tile.TileContext(nc) as tc, contextlib.ExitStack() as ctx:
        pp = ctx.enter_context(tc.tile_pool(name="persist", bufs=1))
        xp = ctx.enter_context(tc.tile_pool(name="xp", bufs=2))
        sbv = ctx.enter_context(tc.tile_pool(name="sbv", bufs=2))
        sbe = ctx.enter_context(tc.tile_pool(name="sbe", bufs=3))
        sba = ctx.enter_context(tc.tile_pool(name="sba", bufs=3))
        sbg = ctx.enter_context(tc.tile_pool(name="sbg", bufs=2))
        sbr = ctx.enter_context(tc.tile_pool(name="sbr", bufs=1))
        sby = ctx.enter_context(tc.tile_pool(name="sby", bufs=2))
        psw = ctx.enter_context(tc.tile_pool(name="psw", bufs=2, space="PSUM"))
        pacc = ctx.enter_context(tc.tile_pool(name="pacc", bufs=2, space="PSUM"))
        psy = ctx.enter_context(tc.tile_pool(name="psy", bufs=1, space="PSUM"))

        qT = pp.tile([CH, BT], F32R, tag="qT")
        kT = pp.tile([CH, BT], F32R, tag="kT")
        wq_sb = pp.tile([128, 8, 128], F32R, tag="wq")
        wk_sb = pp.tile([128, 8, 128], F32R, tag="wk")
        wv_sb = pp.tile([128, 8, 128], F32R, tag="wv")
        wo_sb = pp.tile([CH, D], F32R, tag="wo")
        ident = pp.tile([128, 128], F32, tag="ident")
        hmask_sb = pp.tile([HPC, 128], F32R, tag="hmask")
        v_aug = pp.tile([128, NBKT, 2 * (HS + 1)], F32R, tag="v_aug")
        ones_sb = pp.tile([128, NBKT], F32, tag="ones")
        negLnS = pp.tile([128, NQT, B, HPC], F32, tag="negLnS")
        ycT0 = sbr.tile([CH, T], F32R, tag="ycT0", name="ycT0")
        ycT1 = sbr.tile([CH, T], F32R, tag="ycT1", name="ycT1")
        S_b = sbr.tile([HPC, T], F32, tag="Sb", name="S_b")
        recipSr_b = sbr.tile([HPC, T], F32R, tag="recipSb", name="recipSr_b")
        rawST = sbr.tile([128, NQT, HPC], F32, tag="rawST", name="rawST")
        ycT_b = {0: ycT0, 1: ycT1}

        make_identity(nc, ident[:])
        nc.sync.dma_start(out=hmask_sb, in_=hmask[:, :])
        nc.sync.dma_start(out=wo_sb, in_=woT[:, :])
        for w_sb, w_dr in ((wq_sb, wqT), (wk_sb, wkT), (wv_sb, wvT)):
            nc.sync.dma_start(
                out=w_sb[:],
                in_=w_dr[:, :].rearrange("(dt p) c -> p dt c", p=128),
            )
        nc.vector.memset(ones_sb[:], 1.0)
        nc.vector.tensor_copy(v_aug[:, :, HS], ones_sb[:])
        nc.vector.tensor_copy(v_aug[:, :, 2 * HS + 1], ones_sb[:])

        # x DMAs all emitted upfront; xp bufs=2 paces them against consumption.
        x_tiles = []
        for e in range(8):
            x_sb = xp.tile([128, 8, 512], F32R, tag="xq", name=f"x{e}")
            for dt in range(8):
                nc.sync.dma_start(
                    out=x_sb[:, dt, :],
                    in_=xT[dt * 128:(dt + 1) * 128, e * 512:(e + 1) * 512],
                )
            x_tiles.append(x_sb)

        vT_es = {}

        def proj_group(e, w_sb, dest, off):
            def f():
                acc = pacc.tile([128, 512], F32, tag="acc", name="acc")
                for dt in range(8):
                    nc.tensor.matmul(
                        acc[:],
                        w_sb[:, dt, :],
                        x_tiles[e][:, dt, :],
                        start=(dt == 0),
                        stop=(dt == 7),
                    )
                nc.vector.tensor_copy(dest[:, off:off + 512], acc[:])
            return f

        def trans_group(e):
            def f():
                vT_e = vT_es[e]
                for j in range(4):
                    bkt = e * 4 + j
                    tp = pacc.tile([128, 512], F32, tag="acc", name="tp")
                    nc.tensor.transpose(
                        tp[:, 0:128], vT_e[:, ts(j, 128)].bitcast(F32), ident[:]
                    )
                    nc.vector.tensor_copy(v_aug[:, bkt, 0:HS], tp[:, 0:HS])
                    nc.vector.tensor_copy(
                        v_aug[:, bkt, HS + 1:2 * HS + 1], tp[:, HS:2 * HS]
                    )
            return f

        def eighth_fillers(e):
            vT_es[e] = sbv.tile([CH, 512], F32R, tag="vTe", name=f"vTe{e}")
            return [
                proj_group(e, wq_sb, qT, e * 512),
                proj_group(e, wk_sb, kT, e * 512),
                proj_group(e, wv_sb, vT_es[e], 0),
                trans_group(e),
            ]

        def emit_pass_a(b):
            """Heads packed in PE row groups: unnormalized y + S (+S transposes)."""
            ycT = ycT_b[b]
            pending_strans = []

            def strans(qts):
                def f():
                    for qt in qts:
                        stp = pacc.tile([128, 512], F32, tag="acc", name="stp")
                        nc.tensor.transpose(
                            stp[:, 0:HPC],
                            S_b[0:HPC, qt * 128:(qt + 1) * 128],
                            ident[0:HPC, 0:HPC],
                        )
                        nc.vector.tensor_copy(rawST[:, qt, :], stp[:, 0:HPC])
                return f

            for qc in range(4):  # 512-wide q chunks
                q0 = b * T + qc * 512
                yT0 = psy.tile([HS + 1, 512], F32, tag="yT0", name="yT0")
                yT1 = psy.tile([HS + 1, 512], F32, tag="yT1", name="yT1")
                for kt in range(NKT):
                    if kt == 2 and pending_strans:
                        pending_strans.pop(0)()
                    k0 = b * T + kt * 128
                    wei = psw.tile([128, 1024], F32, tag="w")
                    nc.tensor.matmul(
                        wei[:, 0:512],
                        kT[0:HS, k0:k0 + 128],
                        qT[0:HS, q0:q0 + 512],
                        start=True, stop=True,
                    )
                    nc.tensor.matmul(
                        wei[:, 512:1024],
                        kT[HS:2 * HS, k0:k0 + 128],
                        qT[HS:2 * HS, q0:q0 + 512],
                        start=True, stop=True,
                    )
                    expT = sbe.tile([128, 1024], F32R, tag="expT")
                    nc.scalar.activation(expT[:], wei[:], Exp)
                    nc.tensor.matmul(
                        yT0[:],
                        v_aug[:, b * NKT + kt, 0:HS + 1],
                        expT[:, 0:512],
                        start=(kt == 0), stop=(kt == NKT - 1),
                    )
                    nc.tensor.matmul(
                        yT1[:],
                        v_aug[:, b * NKT + kt, HS + 1:2 * (HS + 1)],
                        expT[:, 512:1024],
                        start=(kt == 0), stop=(kt == NKT - 1),
                    )
                for h, yT_ps in ((0, yT0), (1, yT1)):
                    stage65 = sbg.tile([HS + 1, 512], F32, tag="st65")
                    nc.vector.tensor_copy(stage65[:], yT_ps[:])
                    if h == 0:
                        nc.vector.tensor_copy(
                            ycT[0:HS, qc * 512:(qc + 1) * 512], yT_ps[0:HS, :]
                        )
                    else:
                        nc.gpsimd.dma_start(
                            out=ycT[HS:2 * HS, qc * 512:(qc + 1) * 512],
                            in_=stage65[0:HS, :],
                        )
                    nc.gpsimd.dma_start(
                        out=S_b[h:h + 1, qc * 512:(qc + 1) * 512],
                        in_=stage65[HS:HS + 1, :],
                    )
                if qc < 3:
                    pending_strans.append(strans(range(qc * 4, qc * 4 + 4)))
                else:
                    strans(range(12, 16))()
            for f in pending_strans:
                f()

        def emit_sprep_c(b):
            """-ln(S) bias prep + 1/S scaling of ycT (recipS = exp(-ln S) on ACT)."""
            ycT = ycT_b[b]
            nc.scalar.activation(negLnS[:, :, b, :], rawST[:], Ln)
            lnS_b = sba.tile([HPC, T], F32, tag="att", name="lnS_b")
            nc.scalar.activation(lnS_b[:], S_b[:], Ln)
            nc.vector.tensor_scalar_mul(
                negLnS[:, :, b, :], negLnS[:, :, b, :], -1.0
            )
            nc.scalar.activation(recipSr_b[:], lnS_b[:], Exp, scale=-1.0)
            for c2 in range(2):  # 1024-token chunks within b
                bc = psw.tile([128, 1024], F32, tag="w")
                for n in range(2):
                    nc.tensor.matmul(
                        bc[:, ts(n, 512)],
                        hmask_sb[:],
                        recipSr_b[0:HPC, c2 * 1024 + n * 512:
                                  c2 * 1024 + (n + 1) * 512],
                        start=True, stop=True,
                    )
                nc.vector.tensor_mul(
                    ycT[:, ts(c2, 1024)], ycT[:, ts(c2, 1024)], bc[:]
                )

        def emit_pass_b(b, fillers):
            """Normalized att via exp(wei - lnS); out-projection interleaved."""
            ycT = ycT_b[b]
            for qt in range(NQT):
                qs = b * T + qt * 128
                att0 = sba.tile([128, T], F32, tag="att")
                att1 = sba.tile([128, T], F32, tag="att")
                for kc in range(2):  # 1024-wide k chunks
                    wei0 = psw.tile([128, 1024], F32, tag="w")
                    wei1 = psw.tile([128, 1024], F32, tag="w")
                    for n in range(2):
                        ks = b * T + kc * 1024 + n * 512
                        nc.tensor.matmul(
                            wei0[:, ts(n, 512)],
                            qT[0:HS, qs:qs + 128],
                            kT[0:HS, ks:ks + 512],
                            start=True, stop=True,
                        )
                        nc.tensor.matmul(
                            wei1[:, ts(n, 512)],
                            qT[HS:2 * HS, qs:qs + 128],
                            kT[HS:2 * HS, ks:ks + 512],
                            start=True, stop=True,
                        )
                    nc.scalar.activation(
                        att0[:, ts(kc, 1024)], wei0[:], Exp,
                        bias=negLnS[:, qt, b, 0:1],
                    )
                    nc.scalar.activation(
                        att1[:, ts(kc, 1024)], wei1[:], Exp,
                        bias=negLnS[:, qt, b, 1:2],
                    )
                    if kc == 0 and fillers:
                        fillers.pop(0)()
                nc.sync.dma_start(out=att_out[b, 0, ts(qt, 128), :], in_=att0[:])
                nc.sync.dma_start(out=att_out[b, 1, ts(qt, 128), :], in_=att1[:])
                # out-projection for token tile qt of this b (psum via psy slots)
                tok = b * T + qt * 128
                op0 = psy.tile([128, 512], F32, tag="yT0", name="op0")
                op1 = psy.tile([128, 512], F32, tag="yT1", name="op1")
                nc.tensor.matmul(
                    op0[:], ycT[:, qt * 128:(qt + 1) * 128], wo_sb[:, 0:512],
                    start=True, stop=True,
                )
                nc.tensor.matmul(
                    op1[:], ycT[:, qt * 128:(qt + 1) * 128], wo_sb[:, 512:1024],
                    start=True, stop=True,
                )
                y_sb = sby.tile([128, D], F32, tag="y")
                nc.vector.tensor_copy(y_sb[:, 0:512], op0[:])
                nc.vector.tensor_copy(y_sb[:, 512:1024], op1[:])
                nc.sync.dma_start(out=y_out[tok:tok + 128, :], in_=y_sb[:])

        # ---- software-pipelined emission ----
        for e in range(4):           # b=0 inputs inline
            for f in eighth_fillers(e):
                f()
        late = []
        for e in range(4, 8):        # b=1 inputs fill pass B of b=0
            late.extend(eighth_fillers(e))
        emit_pass_a(0)
        emit_sprep_c(0)
        emit_pass_b(0, late)
        assert not late
        emit_pass_a(1)
        emit_sprep_c(1)
        emit_pass_b(1, [])

    nc.finalize()
    return nc


def kernel(x, Wq, Wk, Wv, Wo, bo, _trace=False, _tmpdir=None):
    global LAST_EXEC_NS, LAST_RESULT, _NC_CACHE
    x = np.asarray(x, dtype=np.float32)
    Wq = np.asarray(Wq, dtype=np.float32)
    Wk = np.asarray(Wk, dtype=np.float32)
    Wv = np.asarray(Wv, dtype=np.float32)
    Wo = np.asarray(Wo, dtype=np.float32)
    bo = np.asarray(bo, dtype=np.float32)

    scale = 1.0 / np.sqrt(np.float32(HS))
    xT_host = np.ascontiguousarray(x.reshape(BT, D).T)
    hm = np.zeros((HPC, 128), np.float32)
    for h in range(HPC):
        hm[h, h * HS:(h + 1) * HS] = 1.0

    in_maps = []
    for c in range(NCORES):
        cs = slice(c * CH, (c + 1) * CH)
        in_maps.append({
            "xT": xT_host,
            "wqT": np.ascontiguousarray(Wq[cs, :].T) * scale,
            "wkT": np.ascontiguousarray(Wk[cs, :].T),
            "wvT": np.ascontiguousarray(Wv[cs, :].T),
            "woT": np.ascontiguousarray(Wo[:, cs].T),
            "hmask": hm,
        })

    if _NC_CACHE is None:
        _NC_CACHE = _build()
    nc = _NC_CACHE

    res = run_bass_kernel_spmd(
        nc, in_maps, core_ids=list(range(NCORES)), trace=_trace, tmpdir=_tmpdir
    )
    LAST_EXEC_NS = res.exec_time_ns
    LAST_RESULT = res

    att = np.empty((B, H, T, T), dtype=np.float32)
    y = np.zeros((BT, D), dtype=np.float64)
    for c in range(NCORES):
        att[:, c * HPC:(c + 1) * HPC] = res.results[c]["att"]
        y += res.results[c]["y"]
    y = (y + bo).astype(np.float32).reshape(B, T, D)
    return y, att
